# revision 79
# baseline (speedup 1.0000x reference)
"""MultiHeadAttention Trainium2 kernel (8 NeuronCores, SPMD).

Reference computation (B=4, T=1024, D=768, H=12, Dh=64):
    q = x @ Wq.T ; k = x @ Wk.T ; v = x @ Wv.T       (per-head reshape)
    attn = softmax((q @ k.T) / 8)
    out = (attn @ v) @ Wo.T + bo

Sharding: 8 cores = 4 batches x 2 head-halves (6 heads each). Each core
computes a [1024, 768] partial of the output projection for its 6 heads;
the host sums the two partials per batch and adds the bias.

fp8 DoubleRow strategy (DR contracts [p<=128, 2] free-dim pairs at 0.5
cycles/output-column -- 4x the MAC throughput of fp32r/bf16 per pass):
  - Projections: x and W*64 pre-split on host into e4m3 (hi, lo) with
    lo = fp8(a - fp8(a)), DR-contracted over d_in chunk pairs. v uses
    three compensation terms (hi*hi + lo*hi + hi*lo, ~bf16 accuracy at
    0.75x the fp32r PE cost); q/k drop the x_lo term (0.5x cost) since
    k is requantized to single fp8 for the scores matmul anyway.
  - Scores: DR with the moving operand carrying the (q_hi, q_lo)
    compensation pair and the stationary side a duplicated single-fp8 k.
    S = k8 . (q_hi + q_lo): half the PE cost, error ~ k/x quantization
    (7.5e-3 rel fro end to end on hardware, vs the 2e-2 gate).
  - exp folds the 64*64 operand scaling away via its scale arg
    (S_psum = 32768 * logits); the ones column is 64.0 so the softmax
    denominator in the ctx matmul carries the same scale as 64*v.
  - ctx + out_proj run in fp16 (same 1.0 cycles/row, half the SBUF/DMA);
    the out partials leave as fp16 too (|out| ~ 0.1, fp16 err ~5e-4).

The ctx stage is computed in the FLIPPED layout out[q_tile(128), 65]:
the matmul cost model charges only the streamed dimension, so streaming
the 64 v columns + denominator column (65) instead of 512 q columns
halves the ctx PE cost, and the softmax denominator lands as a
per-partition scalar (DVE reciprocal + tensor_scalar_mul -- no gpsimd
partition broadcast). PE transposes (identity matmul, fp16 psum, odd
head via the col-64 tile position) rebuild the ctxT layout the output
projection needs, one [128,512] psum + single 2x-mode DVE copy per
head-pair/qt.

Schedule: the 48 scores-tile units (2 DR matmuls + 1 exp each) alternate
1:1 with filler units (projection groups, flipped-ctx units, out tiles)
in dependency order, because the 2-deep scores psum pool ties the PE
scores stream to ~2 exps ahead of ACT and PE's queue is in-order. Each
flipped-ctx unit holds all four of its head's expS tiles until it runs,
so the expS pool is 22 deep (two full phases + lead). qk m1 emits in
the head shadow and all non-critical DMA issues ride the SP sequencer
(ACT's sequencer must stay clear to decode the first exps). A
dummy-matmul chain bridges the initial DMA wait so the cost model's PE
p-state ramp elapses on throwaway work; after the final exp only head
B's four ctx units, one transpose batch, and the qt1 out tiles
(borrowing the dead scores psum banks) remain.

Per-core dataflow:
    x_hi/x_lo [768,1024] e4m3 (host-split, pretransposed) -> SBUF
    q',k' = 64*(x W) via 6 DR matmuls per [128,512] psum tile (v: 9)
        q8 = (fp8(q'), fp8(q'-fp8(q'))) pair; k8 duplicated pair
    S.T tiles [kt=128, qt=512] = DR(k8_pair, q8_pair)
    expS = exp(S.T * 1/32768) via ScalarE -> fp16
    ctx psum [q 128, 65] = expST.T @ [64v16 | 64.0]  (fp16, K=kt accum)
    ctx_norm = ctx * recip(col 64) -> fp16, PE-transposed back to ctxT
    out_partial = ctxT_norm.T @ Wo16.T   (fp16, K=384 accumulate) -> fp16
"""

import numpy as np
import ml_dtypes

import concourse.mybir as mybir
from concourse import bacc
from concourse.tile import TileContext
from concourse.bass_utils import run_bass_kernel_spmd

FP = mybir.dt.float32
F16 = mybir.dt.float16
F8 = mybir.dt.float8e4
AF = mybir.ActivationFunctionType
DR = mybir.MatmulPerfMode.DoubleRow

E4NP = ml_dtypes.float8_e4m3

B, T, D = 4, 1024, 768
H, DH = 12, 64
NCORES = 8
HPC = 6           # heads per core
DPC = HPC * DH    # 384 head-dims per core
KC = D // 128     # 6 contraction chunks for d_in
CP = KC // 2      # 3 chunk-pairs for DoubleRow
MC = DPC // 128   # 3 chunks of per-core head dims
NT = T // 512     # 2 free-dim tiles of tokens
TT = T // 128     # 8 partition tiles of tokens

WSCALE = 64.0             # host scale on Wq/Wk/Wv (fp8 range usage)
EXP_SCALE = 1.0 / 32768.0  # exp reads S_psum = 64q . 64k = 32768*(qk/8)
ONES_VAL = 64.0           # denominator column matches the 64*v scale


def emit_mha(tc, xh, xl, wqh, wql, wkh, wkl, wvh, wvl, wo, ones, ident, out, ctx):
    nc = tc.nc

    singles = ctx.enter_context(tc.tile_pool(name="singles", bufs=1))
    proj_psum = ctx.enter_context(tc.tile_pool(name="proj_psum", bufs=2, space="PSUM"))
    scores_psum = ctx.enter_context(
        tc.tile_pool(name="scores_psum", bufs=2, space="PSUM")
    )
    ctx_psum = ctx.enter_context(tc.tile_pool(name="ctx_psum", bufs=2, space="PSUM"))
    expS_pool = ctx.enter_context(tc.tile_pool(name="expS", bufs=22))
    rcp_pool = ctx.enter_context(tc.tile_pool(name="rcp", bufs=10))
    ctxN_pool = ctx.enter_context(tc.tile_pool(name="ctxN", bufs=12))
    out_pool = ctx.enter_context(tc.tile_pool(name="outsb", bufs=6))

    # ---------------- staged input DMAs ----------------
    xh_sb = singles.tile([128, KC, T], F8, name="xh_sb", tag="xh_sb")
    xl_sb = singles.tile([128, KC, T], F8, name="xl_sb", tag="xl_sb")
    xhr = xh.rearrange("(c p) t -> p c t", p=128)
    xlr = xl.rearrange("(c p) t -> p c t", p=128)
    w_sb = {}
    w_r = {}
    for nm, t in (("qh", wqh), ("ql", wql), ("kh", wkh), ("kl", wkl),
                  ("vh", wvh), ("vl", wvl)):
        w_sb[nm] = singles.tile([128, KC, DPC], F8, name=f"w{nm}", tag=f"w{nm}")
        w_r[nm] = t.rearrange("(c p) d -> p c d", p=128)
    wo_sb = singles.tile([128, MC, D], F16, name="wo_sb", tag="wo_sb")

    # Few, large DMAs (each dma_start costs ~600ns of serial issue time on
    # its engine's sequencer). The qk-m0/n0-critical transfers are split
    # across the idle sequencers so they all issue within ~2 rounds; the x
    # halves are further split so their transfer time halves.
    # The transfer device is effectively serial in the cost model, so the
    # queue ORDER is the arrival order: everything the first two psum
    # groups need goes first, ordered by when the group's matmuls touch
    # it (x_lo last: the hi*lo/lo*hi terms are emitted last). Issue
    # alternates SP/ACT sequencers to keep the queue fed.
    ones_sb = singles.tile([128, HPC], F16, name="ones_sb", tag="ones_sb")
    # with 2-term q/k, x_lo is only needed by the v projection -- the
    # critical head chain is just x_hi n0 + the four qk m0 weight slices
    nc.sync.dma_start(out=xh_sb[:, :, 0:512], in_=xhr[:, :, 0:512])
    nc.scalar.dma_start(out=w_sb["qh"][:, :, 0:128], in_=w_r["qh"][:, :, 0:128])
    nc.sync.dma_start(out=w_sb["ql"][:, :, 0:128], in_=w_r["ql"][:, :, 0:128])
    nc.scalar.dma_start(out=w_sb["kh"][:, :, 0:128], in_=w_r["kh"][:, :, 0:128])
    nc.sync.dma_start(out=w_sb["kl"][:, :, 0:128], in_=w_r["kl"][:, :, 0:128])
    nc.scalar.dma_start(out=xh_sb[:, :, 512:1024], in_=xhr[:, :, 512:1024])
    nc.gpsimd.dma_start(out=ones_sb, in_=ones)
    # everything below is issue-latency-insensitive: it all goes on the SP
    # sequencer so the ACT sequencer is free to decode the first exps
    # (each dma_start occupies its sequencer ~650ns, and ACT's queue sits
    # ahead of every activation instruction)
    for nm in ("kh", "qh", "kl", "ql"):
        nc.sync.dma_start(out=w_sb[nm][:, :, 128:256], in_=w_r[nm][:, :, 128:256])
    for nm in ("kh", "qh", "kl", "ql"):
        nc.sync.dma_start(out=w_sb[nm][:, :, 256:DPC], in_=w_r[nm][:, :, 256:DPC])
    nc.sync.dma_start(out=w_sb["vh"], in_=w_r["vh"])
    nc.sync.dma_start(out=w_sb["vl"], in_=w_r["vl"])
    nc.sync.dma_start(out=xl_sb[:, :, 0:512], in_=xlr[:, :, 0:512])
    nc.sync.dma_start(out=xl_sb[:, :, 512:1024], in_=xlr[:, :, 512:1024])
    nc.sync.dma_start(out=wo_sb, in_=wo.rearrange("(c p) d -> p c d", p=128))
    ident_sb = singles.tile([128, 128], F16, name="ident_sb", tag="ident_sb")
    nc.sync.dma_start(out=ident_sb, in_=ident)

    # warm-up: a chain of dummy matmuls keeps PE continuously busy from
    # ~1.2us until the first inputs land (~4.5us), so the cost model's
    # 3us p-state ramp elapses on throwaway work. The ramp clock resets
    # whenever PE goes idle, so the chain must bridge the whole DMA wait.
    wu_sb = singles.tile([128, 256], F16, name="wu_sb", tag="wu_sb")
    nc.vector.memset(wu_sb, 0.0)
    for _ in range(5):
        ps_wu = proj_psum.tile([128, 512], FP, name="ps_wu", tag="proj")
        nc.tensor.matmul(ps_wu[:, 0:256], lhsT=wu_sb[:, 0:128],
                         rhs=wu_sb[:, 0:256], start=True, stop=True)
        nc.tensor.matmul(ps_wu[:, 256:512], lhsT=wu_sb[:, 0:128],
                         rhs=wu_sb[:, 0:256], start=True, stop=True)

    # q8: (hi, lo) compensation pair; k8: duplicated single-fp8 pair
    q8_sb = singles.tile([128, MC, 2, T], F8, name="q8_sb", tag="q8_sb")
    k8_sb = singles.tile([128, MC, 2, T], F8, name="k8_sb", tag="k8_sb")
    ctxT_sb = singles.tile([128, MC, T], F16, name="ctxT_sb", tag="ctxT_sb")

    # v tiles [t_tile, 6 heads x (64 v cols + ones col)]: the 64.0 column
    # makes each head's ctx matmul also produce its softmax denominator
    # (psum row 64) at the same 64x scale as v. One early DMA stages the
    # column; gpsimd copies fan it out (NOT on DVE, whose in-order queue
    # must stay clear for the q8/k8 psum copies).
    v_sb = []
    for i in range(TT):
        vt = singles.tile([128, HPC, DH + 1], F16, name=f"v_sb{i}", tag=f"v_sb{i}")
        nc.gpsimd.tensor_copy(vt[:, :, DH : DH + 1], ones_sb)
        v_sb.append(vt)

    # DR terms of the hi/lo-compensated product, as (x, w) suffixes.
    # v keeps all three terms; q/k drop the x_lo correction -- k is about
    # to be requantized to single fp8 for the scores matmul anyway and q's
    # error stays ~fp8-lo sized (1.1e-2 rel fro end to end, vs 2e-2 gate).
    TERMS3 = (("h", "h"), ("h", "l"), ("l", "h"))
    TERMS2 = (("h", "h"), ("h", "l"))

    def qk_proj(m, ns=range(NT), dsts=("q", "k"), fine=False, k_on_act=False):
        # q'/k' chunk m: psum[m=dout(128), n=t(512)] = 64 * sum_c w[c].T x[c]
        # 9 DR matmuls: 3 hi/lo terms x 3 chunk-pairs, each contracting 256
        xs = {"h": xh_sb, "l": xl_sb}
        paths = {"k": (("kh", "kl"), k8_sb, False), "q": (("qh", "ql"), q8_sb, True)}
        for n in ns:
            for wk_, dst, is_q in (paths[d] for d in dsts):
                ps = proj_psum.tile([128, 512], FP, name="ps_qk", tag="proj")
                first = True
                for xsfx, wsfx in TERMS2:
                    wt = w_sb[wk_[0][0] + wsfx]
                    xt = xs[xsfx]
                    for cp in range(CP):
                        nc.tensor.matmul(
                            ps,
                            lhsT=wt[:, 2 * cp : 2 * cp + 2, m * 128 : (m + 1) * 128],
                            rhs=xt[:, 2 * cp : 2 * cp + 2, n * 512 : (n + 1) * 512],
                            start=first,
                            stop=(xsfx, wsfx) == TERMS2[-1] and cp == CP - 1,
                            perf_mode=DR,
                        )
                        first = False
                # fine=True splits the k copies in half-widths so the first
                # scores tiles (which only need k columns 0:256) unblock
                # earlier -- used for the critical m0/n0 group only. The q
                # copies stay full-width (every scores tile reads all 512).
                cw = 256 if (fine and not is_q) else 512
                for c0 in range(n * 512, (n + 1) * 512, cw):
                    csl = slice(c0, c0 + cw)
                    psl = slice(c0 - n * 512, c0 - n * 512 + cw)
                    if is_q:
                        nc.vector.tensor_copy(dst[:, m, 0, csl], ps[:, psl])
                        nc.vector.tensor_sub(dst[:, m, 1, csl], ps[:, psl],
                                             dst[:, m, 0, csl])
                    elif k_on_act:
                        # ACT is idle during the head; copying k there runs
                        # in parallel with the q copies on DVE
                        nc.scalar.copy(dst[:, m, 0, csl], ps[:, psl])
                        nc.scalar.copy(dst[:, m, 1, csl], ps[:, psl])
                    else:
                        nc.vector.tensor_copy(dst[:, m, 0, csl], ps[:, psl])
                        nc.vector.tensor_copy(dst[:, m, 1, csl], ps[:, psl])

    def v_proj(mts=range(TT)):
        # v': psum[m=t_tile(128), n=dh(384)] = 64 * sum_c x[c,m].T wv[c,n]
        xs = {"h": xh_sb, "l": xl_sb}
        for mt in mts:
            ps = proj_psum.tile([128, DPC], FP, name="ps_v", tag="proj")
            first = True
            for xsfx, wsfx in TERMS3:
                xt = xs[xsfx]
                wt = w_sb["v" + wsfx]
                for cp in range(CP):
                    nc.tensor.matmul(
                        ps,
                        lhsT=xt[:, 2 * cp : 2 * cp + 2, mt * 128 : (mt + 1) * 128],
                        rhs=wt[:, 2 * cp : 2 * cp + 2, :],
                        start=first,
                        stop=(xsfx, wsfx) == TERMS3[-1] and cp == CP - 1,
                        perf_mode=DR,
                    )
                    first = False
            nc.vector.tensor_copy(v_sb[mt][:, :, 0:DH], ps)

    def mk_pair(hp):
        return [(2 * hp, 0, []), (2 * hp + 1, 64, [])]

    def scores_unit(hp, qt, pair, g, hi):
        # one scores psum tile for head pair[hi], k-tiles 2g/2g+1:
        # stationary = duplicated k8 pair [64,2,128], moving = (q_hi, q_lo)
        # compensation pair [64,2,512]; exp follows immediately.
        h, po, exps = pair[hi]
        ps = scores_psum.tile([128, 1024], FP, name="ps_s", tag="scores")
        for r2 in range(2):
            j = 2 * g + r2
            nc.tensor.matmul(
                ps[:, r2 * 512 : (r2 + 1) * 512],
                lhsT=k8_sb[po : po + 64, hp, :, j * 128 : (j + 1) * 128],
                rhs=q8_sb[po : po + 64, hp, :, qt * 512 : (qt + 1) * 512],
                start=True,
                stop=True,
                perf_mode=DR,
            )
        ex = expS_pool.tile([128, 1024], F16, name="ex", tag="expS")
        nc.scalar.activation(ex, ps, AF.Exp, scale=EXP_SCALE)
        exps.append(ex)

    def ctx_q(hp, qt, pair, hi, qs):
        # flipped ctx: out[q_tile(128), 65] = sum_kt expST[kt, q].T @ [64v|64]
        # -- the streamed dim is only 65 wide (half the PE cost of the
        # [65, 512] layout) and the denominator lands as column 64, a
        # per-partition scalar: recip + tensor_scalar_mul, no broadcast.
        h, po, exps = pair[hi]
        pc = ctx_psum.tile([128, 65], FP, name="pcq", tag="ctx")
        c0 = qs * 128
        for j in range(TT):
            nc.tensor.matmul(
                pc,
                lhsT=exps[j // 2][:, (j % 2) * 512 + c0 : (j % 2) * 512 + c0 + 128],
                rhs=v_sb[j][:, h, :],
                start=(j == 0),
                stop=(j == TT - 1),
            )
        rcp = rcp_pool.tile([128, 1], FP, name="rcp", tag="rcp")
        cn = ctxN_pool.tile([128, DH], F16, name="ctxN", tag="ctxN")
        nc.vector.reciprocal(rcp, pc[:, DH : DH + 1])
        nc.vector.tensor_scalar_mul(cn, pc[:, 0:DH], rcp)
        return cn

    def ctx_t(hp, qt, cns):
        # transpose the pair's eight [q(128), dh(64)] normalized tiles back
        # into ctxT layout: PE transposes into one fp16 psum (odd head via
        # the col-64 tile position), then a single 2x-mode DVE copy.
        pt = ctx_psum.tile([128, 512], F16, name="pt", tag="ctx")
        for hi in range(2):
            po = 64 * hi
            for qs in range(4):
                nc.tensor.transpose(
                    pt[po : po + 64, qs * 128 : (qs + 1) * 128],
                    cns[hi][qs],
                    ident_sb,
                )
        nc.vector.tensor_copy(
            ctxT_sb[:, hp, qt * 512 : (qt + 1) * 512], pt)

    cn_store = {}

    def ctx_qu(key, hp, qt, pair, hi, qs):
        cn_store.setdefault(key, [[None] * 4 for _ in range(2)])
        cn_store[key][hi][qs] = ctx_q(hp, qt, pair, hi, qs)

    def ctx_tu(key, hp, qt):
        ctx_t(hp, qt, cn_store[key])

    def ctx_phase(key, hp, qt, pair):
        for hi in range(2):
            for qs in range(4):
                ctx_qu(key, hp, qt, pair, hi, qs)
        ctx_tu(key, hp, qt)

    def mk_ctx_units(key, hp, qt, pair):
        us = [
            (lambda hi=hi, qs=qs: ctx_qu(key, hp, qt, pair, hi, qs))
            for hi in range(2) for qs in range(4)
        ]
        us.append(lambda: ctx_tu(key, hp, qt))
        return us

    def out_proj(mts, split_dma=False):
        # out[m=t_tile(128), n=dout(384)] = sum_c ctxT16[c,m].T @ wo16[c,n];
        # the two psum->sbuf copies split across DVE and ScalarE (ACT is
        # idle by this phase) so the final DMAs unblock sooner. Tail tiles
        # borrow the (dead by then) scores psum pool for their second half
        # so the copy latency stops gating the 2-deep proj rotation.
        for mt in mts:
            osb = out_pool.tile([128, D], F16, name="osb", tag="outsb")
            for n2 in range(2):
                pool = scores_psum if (split_dma and n2 == 1) else proj_psum
                ps = pool.tile([128, 384], FP, name="ps_o", tag="proj" if pool is proj_psum else "scores")
                for c in range(MC):
                    nc.tensor.matmul(
                        ps,
                        lhsT=ctxT_sb[:, c, mt * 128 : (mt + 1) * 128],
                        rhs=wo_sb[:, c, n2 * 384 : (n2 + 1) * 384],
                        start=(c == 0),
                        stop=(c == MC - 1),
                    )
                if n2 == 0:
                    nc.vector.tensor_copy(osb[:, 0:384], ps)
                elif split_dma:
                    # tail tiles: ACT is free (exps done); halves copy in
                    # parallel on DVE + ACT so the tile's DMA unblocks fast
                    nc.scalar.copy(osb[:, 384:768], ps)
                else:
                    # early tiles: exps still stream on ACT, and gpsimd
                    # cannot read PSUM, so DVE takes both halves
                    nc.vector.tensor_copy(osb[:, 384:768], ps)
            # alternate issue sequencers so tail DMAs don't serialize
            eng = nc.sync if mt % 2 == 0 else nc.scalar
            eng.dma_start(out=out[mt * 128 : (mt + 1) * 128, :], in_=osb)

    # The scores psum pool is 2 tiles deep, so the PE scores stream runs
    # exactly ~2 exps ahead of ACT and each scores unit (2 DR matmuls,
    # ~214ns) must be followed by ~0.8us of OTHER ready PE work or PE
    # head-of-line blocks on the pool rotation. The emission is therefore
    # a 1:1 alternation of the 48 scores units with ~40 filler units
    # (projection groups / ctx chunks / out tiles), ordered so every
    # filler's dependencies are already satisfied when it pops and every
    # scores phase's q/k inputs are fully emitted before its first unit.
    p00, p01 = mk_pair(0), mk_pair(0)
    p10, p11 = mk_pair(1), mk_pair(1)
    p20, p21 = mk_pair(2), mk_pair(2)
    pcs = {k: [None, None] for k in ("00", "01", "10", "11", "20", "21")}

    fillers = [
        lambda: qk_proj(2, ns=[0], dsts=("q",)),
        lambda: qk_proj(2, ns=[0], dsts=("k",)),
        lambda: qk_proj(2, ns=[1], dsts=("k",)),
        lambda: qk_proj(2, ns=[1], dsts=("q",)),
        lambda: v_proj([0]), lambda: v_proj([1]),
        lambda: v_proj([2]), lambda: v_proj([3]),
        lambda: v_proj([4]), lambda: v_proj([5]),
        lambda: v_proj([6]), lambda: v_proj([7]),
        *mk_ctx_units("00", 0, 0, p00),
        *mk_ctx_units("01", 0, 1, p01),
        *mk_ctx_units("10", 1, 0, p10),
        lambda: ctx_qu("11", 1, 1, p11, 0, 0),
    ]

    # head: critical qk m0/n0 with fine-split copies, then the (0,0)
    # scores units; the n1-half's k groups slot in before the g2 units
    # that need them, the (not-yet-needed) q n1-half after
    qk_proj(0, ns=[0], fine=True)
    for g, hi in ((0, 0), (0, 1), (1, 0), (1, 1)):
        scores_unit(0, 0, p00, g, hi)
    qk_proj(0, ns=[1], dsts=("k",))
    scores_unit(0, 0, p00, 2, 0)
    scores_unit(0, 0, p00, 2, 1)
    qk_proj(0, ns=[1], dsts=("q",))
    scores_unit(0, 0, p00, 3, 0)
    scores_unit(0, 0, p00, 3, 1)
    qk_proj(1)

    # paired stream: 40 scores units x 40 fillers
    stream = [(0, 1, p01), (1, 0, p10), (1, 1, p11), (2, 0, p20), (2, 1, p21)]
    fi = iter(fillers)
    for hp, qt, pair in stream:
        for g in range(4):
            for hi in range(2):
                scores_unit(hp, qt, pair, g, hi)
                f = next(fi, None)
                if f is not None:
                    f()
    for f in fi:
        f()

    # drain region (the last phases' exps still streaming on ACT)
    for hi in range(2):
        for qs in range(4):
            if (hi, qs) != (0, 0):
                ctx_qu("11", 1, 1, p11, hi, qs)
    ctx_tu("11", 1, 1)
    ctx_phase("20", 2, 0, p20)
    out_proj([0])
    out_proj([1])
    out_proj([2])
    out_proj([3])
    for qs in range(4):
        ctx_qu("21", 2, 1, p21, 0, qs)
    # true tail: after the final exp only head B's four ctx units, the
    # pair's transpose + copy, and the qt1 out tiles remain
    for qs in range(4):
        ctx_qu("21", 2, 1, p21, 1, qs)
    ctx_tu("21", 2, 1)
    for mt in range(4, TT):
        out_proj([mt], split_dma=True)


_PROGRAM = None


def build_program():
    global _PROGRAM
    if _PROGRAM is not None:
        return _PROGRAM
    nc = bacc.Bacc("TRN2", target_bir_lowering=False, debug=False, num_devices=NCORES)
    xh = nc.dram_tensor("xh", (D, T), F8, kind="ExternalInput").ap()
    xl = nc.dram_tensor("xl", (D, T), F8, kind="ExternalInput").ap()
    wqh = nc.dram_tensor("wqh", (D, DPC), F8, kind="ExternalInput").ap()
    wql = nc.dram_tensor("wql", (D, DPC), F8, kind="ExternalInput").ap()
    wkh = nc.dram_tensor("wkh", (D, DPC), F8, kind="ExternalInput").ap()
    wkl = nc.dram_tensor("wkl", (D, DPC), F8, kind="ExternalInput").ap()
    wvh = nc.dram_tensor("wvh", (D, DPC), F8, kind="ExternalInput").ap()
    wvl = nc.dram_tensor("wvl", (D, DPC), F8, kind="ExternalInput").ap()
    wo = nc.dram_tensor("wo", (DPC, D), F16, kind="ExternalInput").ap()
    ones = nc.dram_tensor("ones", (128, HPC), F16, kind="ExternalInput").ap()
    ident = nc.dram_tensor("ident", (128, 128), F16, kind="ExternalInput").ap()
    out = nc.dram_tensor("out", (T, D), F16, kind="ExternalOutput").ap()
    from contextlib import ExitStack

    with TileContext(nc) as tc, ExitStack() as st:
        emit_mha(tc, xh, xl, wqh, wql, wkh, wkl, wvh, wvl, wo, ones, ident, out, st)
    nc.compile()
    _PROGRAM = nc
    return nc


def _split8(a):
    hi = np.clip(a, -240.0, 240.0).astype(E4NP)
    lo = np.clip(a - hi.astype(np.float32), -240.0, 240.0).astype(E4NP)
    return np.ascontiguousarray(hi), np.ascontiguousarray(lo)


def make_in_maps(x, Wq, Wk, Wv, Wo):
    x = np.asarray(x, dtype=np.float32)
    ones = np.full((128, HPC), ONES_VAL, np.float16)
    ident = np.eye(128, dtype=np.float16)
    xs = [_split8(x[b].T) for b in range(B)]
    in_maps = []
    for core in range(NCORES):
        b, hh = core // 2, core % 2
        sl = slice(hh * DPC, (hh + 1) * DPC)
        wqh, wql = _split8(np.asarray(Wq)[sl].T.astype(np.float32) * WSCALE)
        wkh, wkl = _split8(np.asarray(Wk)[sl].T.astype(np.float32) * WSCALE)
        wvh, wvl = _split8(np.asarray(Wv)[sl].T.astype(np.float32) * WSCALE)
        in_maps.append(
            {
                "xh": xs[b][0],
                "xl": xs[b][1],
                "wqh": wqh, "wql": wql,
                "wkh": wkh, "wkl": wkl,
                "wvh": wvh, "wvl": wvl,
                "wo": np.ascontiguousarray(np.asarray(Wo)[:, sl].T.astype(np.float16)),
                "ones": ones,
                "ident": ident,
            }
        )
    return in_maps


def kernel(x, Wq, Wk, Wv, Wo, bo):
    nc = build_program()
    in_maps = make_in_maps(x, Wq, Wk, Wv, Wo)
    res = run_bass_kernel_spmd(nc, in_maps, core_ids=list(range(NCORES)))
    bo = np.asarray(bo, dtype=np.float32)
    out = np.empty((B, T, D), dtype=np.float32)
    for b in range(B):
        out[b] = (res.results[2 * b]["out"].astype(np.float32)
                  + res.results[2 * b + 1]["out"].astype(np.float32) + bo)
    return out


# revision 84
# speedup vs baseline: 1.0055x; 1.0055x over previous
"""MultiHeadAttention Trainium2 kernel (8 NeuronCores, SPMD).

Reference computation (B=4, T=1024, D=768, H=12, Dh=64):
    q = x @ Wq.T ; k = x @ Wk.T ; v = x @ Wv.T       (per-head reshape)
    attn = softmax((q @ k.T) / 8)
    out = (attn @ v) @ Wo.T + bo

Sharding: 8 cores = 4 batches x 2 head-halves (6 heads each). Each core
computes a [1024, 768] partial of the output projection for its 6 heads;
the host sums the two partials per batch and adds the bias.

fp8 DoubleRow strategy (DR contracts [p<=128, 2] free-dim pairs at 0.5
cycles/output-column -- 4x the MAC throughput of fp32r/bf16 per pass):
  - Projections: x and W*64 pre-split on host into e4m3 (hi, lo) with
    lo = fp8(a - fp8(a)), DR-contracted over d_in chunk pairs. v uses
    three compensation terms (hi*hi + lo*hi + hi*lo, ~bf16 accuracy at
    0.75x the fp32r PE cost); q/k drop the x_lo term (0.5x cost) since
    k is requantized to single fp8 for the scores matmul anyway.
  - Scores: DR with the moving operand carrying the (q_hi, q_lo)
    compensation pair and the stationary side a duplicated single-fp8 k.
    S = k8 . (q_hi + q_lo): half the PE cost, error ~ k/x quantization
    (7.5e-3 rel fro end to end on hardware, vs the 2e-2 gate).
  - exp folds the 64*64 operand scaling away via its scale arg
    (S_psum = 32768 * logits); the ones column is 64.0 so the softmax
    denominator in the ctx matmul carries the same scale as 64*v.
  - ctx + out_proj run in fp16 (same 1.0 cycles/row, half the SBUF/DMA);
    the out partials leave as fp16 too (|out| ~ 0.1, fp16 err ~5e-4).

The ctx stage is computed in the FLIPPED layout out[q_tile(128), 65]:
the matmul cost model charges only the streamed dimension, so streaming
the 64 v columns + denominator column (65) instead of 512 q columns
halves the ctx PE cost, and the softmax denominator lands as a
per-partition scalar (DVE reciprocal + tensor_scalar_mul -- no gpsimd
partition broadcast). PE transposes (identity matmul, fp16 psum, odd
head via the col-64 tile position) rebuild the ctxT layout the output
projection needs, one [128,512] psum + single 2x-mode DVE copy per
head-pair/qt.

Schedule: the 48 scores-tile units (2 DR matmuls + 1 exp each) alternate
1:1 with filler units (projection groups, flipped-ctx units, out tiles)
in dependency order, because the 2-deep scores psum pool ties the PE
scores stream to ~2 exps ahead of ACT and PE's queue is in-order. Each
flipped-ctx unit holds all four of its head's expS tiles until it runs,
so the expS pool is 22 deep (two full phases + lead). qk m1 emits in
the head shadow and all non-critical DMA issues ride the SP sequencer
(ACT's sequencer must stay clear to decode the first exps). A
dummy-matmul chain bridges the initial DMA wait so the cost model's PE
p-state ramp elapses on throwaway work; after the final exp only head
B's four ctx units, one transpose batch, and the qt1 out tiles
(borrowing the dead scores psum banks) remain.

Per-core dataflow:
    x_hi/x_lo [768,1024] e4m3 (host-split, pretransposed) -> SBUF
    q',k' = 64*(x W) via 6 DR matmuls per [128,512] psum tile (v: 9)
        q8 = (fp8(q'), fp8(q'-fp8(q'))) pair; k8 duplicated pair
    S.T tiles [kt=128, qt=512] = DR(k8_pair, q8_pair)
    expS = exp(S.T * 1/32768) via ScalarE -> fp16
    ctx psum [q 128, 65] = expST.T @ [64v16 | 64.0]  (fp16, K=kt accum)
    ctx_norm = ctx * recip(col 64) -> fp16, PE-transposed back to ctxT
    out_partial = ctxT_norm.T @ Wo16.T   (fp16, K=384 accumulate) -> fp16
"""

import numpy as np
import ml_dtypes

import concourse.mybir as mybir
from concourse import bacc
from concourse.tile import TileContext
from concourse.bass_utils import run_bass_kernel_spmd

FP = mybir.dt.float32
F16 = mybir.dt.float16
F8 = mybir.dt.float8e4
AF = mybir.ActivationFunctionType
DR = mybir.MatmulPerfMode.DoubleRow

E4NP = ml_dtypes.float8_e4m3

B, T, D = 4, 1024, 768
H, DH = 12, 64
NCORES = 8
HPC = 6           # heads per core
DPC = HPC * DH    # 384 head-dims per core
KC = D // 128     # 6 contraction chunks for d_in
CP = KC // 2      # 3 chunk-pairs for DoubleRow
MC = DPC // 128   # 3 chunks of per-core head dims
NT = T // 512     # 2 free-dim tiles of tokens
TT = T // 128     # 8 partition tiles of tokens

WSCALE = 64.0             # host scale on Wq/Wk/Wv (fp8 range usage)
EXP_SCALE = 1.0 / 32768.0  # exp reads S_psum = 64q . 64k = 32768*(qk/8)
ONES_VAL = 64.0           # denominator column matches the 64*v scale


def emit_mha(tc, xh, xl, wqh, wql, wkh, wkl, wvh, wvl, wo, ones, ident, out, ctx):
    nc = tc.nc

    singles = ctx.enter_context(tc.tile_pool(name="singles", bufs=1))
    proj_psum = ctx.enter_context(tc.tile_pool(name="proj_psum", bufs=2, space="PSUM"))
    scores_psum = ctx.enter_context(
        tc.tile_pool(name="scores_psum", bufs=2, space="PSUM")
    )
    ctx_psum = ctx.enter_context(tc.tile_pool(name="ctx_psum", bufs=2, space="PSUM"))
    expS_pool = ctx.enter_context(tc.tile_pool(name="expS", bufs=22))
    rcp_pool = ctx.enter_context(tc.tile_pool(name="rcp", bufs=10))
    ctxN_pool = ctx.enter_context(tc.tile_pool(name="ctxN", bufs=12))
    out_pool = ctx.enter_context(tc.tile_pool(name="outsb", bufs=6))

    # ---------------- staged input DMAs ----------------
    xh_sb = singles.tile([128, KC, T], F8, name="xh_sb", tag="xh_sb")
    xl_sb = singles.tile([128, KC, T], F8, name="xl_sb", tag="xl_sb")
    xhr = xh.rearrange("(c p) t -> p c t", p=128)
    xlr = xl.rearrange("(c p) t -> p c t", p=128)
    w_sb = {}
    w_r = {}
    for nm, t in (("qh", wqh), ("ql", wql), ("kh", wkh), ("kl", wkl),
                  ("vh", wvh), ("vl", wvl)):
        w_sb[nm] = singles.tile([128, KC, DPC], F8, name=f"w{nm}", tag=f"w{nm}")
        w_r[nm] = t.rearrange("(c p) d -> p c d", p=128)
    wo_sb = singles.tile([128, MC, D], F16, name="wo_sb", tag="wo_sb")

    # Few, large DMAs (each dma_start costs ~600ns of serial issue time on
    # its engine's sequencer). The qk-m0/n0-critical transfers are split
    # across the idle sequencers so they all issue within ~2 rounds; the x
    # halves are further split so their transfer time halves.
    # The transfer device is effectively serial in the cost model, so the
    # queue ORDER is the arrival order: everything the first two psum
    # groups need goes first, ordered by when the group's matmuls touch
    # it (x_lo last: the hi*lo/lo*hi terms are emitted last). Issue
    # alternates SP/ACT sequencers to keep the queue fed.
    ones_sb = singles.tile([128, HPC], F16, name="ones_sb", tag="ones_sb")
    # with 2-term q/k, x_lo is only needed by the v projection -- the
    # critical head chain is just x_hi n0 + the four qk m0 weight slices
    nc.sync.dma_start(out=xh_sb[:, :, 0:512], in_=xhr[:, :, 0:512])
    nc.scalar.dma_start(out=w_sb["qh"][:, :, 0:128], in_=w_r["qh"][:, :, 0:128])
    nc.sync.dma_start(out=w_sb["ql"][:, :, 0:128], in_=w_r["ql"][:, :, 0:128])
    nc.scalar.dma_start(out=w_sb["kh"][:, :, 0:128], in_=w_r["kh"][:, :, 0:128])
    nc.sync.dma_start(out=w_sb["kl"][:, :, 0:128], in_=w_r["kl"][:, :, 0:128])
    nc.scalar.dma_start(out=xh_sb[:, :, 512:1024], in_=xhr[:, :, 512:1024])
    nc.gpsimd.dma_start(out=ones_sb, in_=ones)
    # everything below is issue-latency-insensitive: it all goes on the SP
    # sequencer so the ACT sequencer is free to decode the first exps
    # (each dma_start occupies its sequencer ~650ns, and ACT's queue sits
    # ahead of every activation instruction)
    for nm in ("kh", "qh", "kl", "ql"):
        nc.sync.dma_start(out=w_sb[nm][:, :, 128:256], in_=w_r[nm][:, :, 128:256])
    for nm in ("kh", "qh", "kl", "ql"):
        nc.sync.dma_start(out=w_sb[nm][:, :, 256:DPC], in_=w_r[nm][:, :, 256:DPC])
    nc.sync.dma_start(out=w_sb["vh"], in_=w_r["vh"])
    nc.sync.dma_start(out=w_sb["vl"], in_=w_r["vl"])
    nc.sync.dma_start(out=xl_sb[:, :, 0:512], in_=xlr[:, :, 0:512])
    nc.sync.dma_start(out=xl_sb[:, :, 512:1024], in_=xlr[:, :, 512:1024])
    nc.sync.dma_start(out=wo_sb, in_=wo.rearrange("(c p) d -> p c d", p=128))
    ident_sb = singles.tile([128, 128], F16, name="ident_sb", tag="ident_sb")
    nc.sync.dma_start(out=ident_sb, in_=ident)

    # warm-up: a chain of dummy matmuls keeps PE continuously busy from
    # ~1.2us until the first inputs land (~4.5us), so the cost model's
    # 3us p-state ramp elapses on throwaway work. The ramp clock resets
    # whenever PE goes idle, so the chain must bridge the whole DMA wait.
    wu_sb = singles.tile([128, 256], F16, name="wu_sb", tag="wu_sb")
    nc.vector.memset(wu_sb, 0.0)
    for _ in range(5):
        ps_wu = proj_psum.tile([128, 512], FP, name="ps_wu", tag="proj")
        nc.tensor.matmul(ps_wu[:, 0:256], lhsT=wu_sb[:, 0:128],
                         rhs=wu_sb[:, 0:256], start=True, stop=True)
        nc.tensor.matmul(ps_wu[:, 256:512], lhsT=wu_sb[:, 0:128],
                         rhs=wu_sb[:, 0:256], start=True, stop=True)

    # q8: (hi, lo) compensation pair; k8: duplicated single-fp8 pair
    q8_sb = singles.tile([128, MC, 2, T], F8, name="q8_sb", tag="q8_sb")
    k8_sb = singles.tile([128, MC, 2, T], F8, name="k8_sb", tag="k8_sb")
    ctxT_sb = singles.tile([128, MC, T], F16, name="ctxT_sb", tag="ctxT_sb")

    # v tiles [t_tile, 6 heads x (64 v cols + ones col)]: the 64.0 column
    # makes each head's ctx matmul also produce its softmax denominator
    # (psum row 64) at the same 64x scale as v. One early DMA stages the
    # column; gpsimd copies fan it out (NOT on DVE, whose in-order queue
    # must stay clear for the q8/k8 psum copies).
    v_sb = []
    for i in range(TT):
        vt = singles.tile([128, HPC, DH + 1], F16, name=f"v_sb{i}", tag=f"v_sb{i}")
        nc.gpsimd.tensor_copy(vt[:, :, DH : DH + 1], ones_sb)
        v_sb.append(vt)

    # DR terms of the hi/lo-compensated product, as (x, w) suffixes.
    # v keeps all three terms; q/k drop the x_lo correction -- k is about
    # to be requantized to single fp8 for the scores matmul anyway and q's
    # error stays ~fp8-lo sized (1.1e-2 rel fro end to end, vs 2e-2 gate).
    TERMS3 = (("h", "h"), ("h", "l"), ("l", "h"))
    TERMS2 = (("h", "h"), ("h", "l"))

    def qk_proj(m, ns=range(NT), dsts=("q", "k"), fine=False, k_on_act=False):
        # q'/k' chunk m: psum[m=dout(128), n=t(512)] = 64 * sum_c w[c].T x[c]
        # 9 DR matmuls: 3 hi/lo terms x 3 chunk-pairs, each contracting 256
        xs = {"h": xh_sb, "l": xl_sb}
        paths = {"k": (("kh", "kl"), k8_sb, False), "q": (("qh", "ql"), q8_sb, True)}
        for n in ns:
            for wk_, dst, is_q in (paths[d] for d in dsts):
                ps = proj_psum.tile([128, 512], FP, name="ps_qk", tag="proj")
                first = True
                for xsfx, wsfx in TERMS2:
                    wt = w_sb[wk_[0][0] + wsfx]
                    xt = xs[xsfx]
                    for cp in range(CP):
                        nc.tensor.matmul(
                            ps,
                            lhsT=wt[:, 2 * cp : 2 * cp + 2, m * 128 : (m + 1) * 128],
                            rhs=xt[:, 2 * cp : 2 * cp + 2, n * 512 : (n + 1) * 512],
                            start=first,
                            stop=(xsfx, wsfx) == TERMS2[-1] and cp == CP - 1,
                            perf_mode=DR,
                        )
                        first = False
                # fine=True splits the k copies in half-widths so the first
                # scores tiles (which only need k columns 0:256) unblock
                # earlier -- used for the critical m0/n0 group only. The q
                # copies stay full-width (every scores tile reads all 512).
                cw = 256 if (fine and not is_q) else 512
                for c0 in range(n * 512, (n + 1) * 512, cw):
                    csl = slice(c0, c0 + cw)
                    psl = slice(c0 - n * 512, c0 - n * 512 + cw)
                    if is_q:
                        nc.vector.tensor_copy(dst[:, m, 0, csl], ps[:, psl])
                        nc.vector.tensor_sub(dst[:, m, 1, csl], ps[:, psl],
                                             dst[:, m, 0, csl])
                    elif k_on_act:
                        # ACT is idle during the head; copying k there runs
                        # in parallel with the q copies on DVE
                        nc.scalar.copy(dst[:, m, 0, csl], ps[:, psl])
                        nc.scalar.copy(dst[:, m, 1, csl], ps[:, psl])
                    else:
                        nc.vector.tensor_copy(dst[:, m, 0, csl], ps[:, psl])
                        nc.vector.tensor_copy(dst[:, m, 1, csl], ps[:, psl])

    def v_proj(mts=range(TT)):
        # v': psum[m=t_tile(128), n=dh(384)] = 64 * sum_c x[c,m].T wv[c,n]
        xs = {"h": xh_sb, "l": xl_sb}
        for mt in mts:
            ps = proj_psum.tile([128, DPC], FP, name="ps_v", tag="proj")
            first = True
            for xsfx, wsfx in TERMS3:
                xt = xs[xsfx]
                wt = w_sb["v" + wsfx]
                for cp in range(CP):
                    nc.tensor.matmul(
                        ps,
                        lhsT=xt[:, 2 * cp : 2 * cp + 2, mt * 128 : (mt + 1) * 128],
                        rhs=wt[:, 2 * cp : 2 * cp + 2, :],
                        start=first,
                        stop=(xsfx, wsfx) == TERMS3[-1] and cp == CP - 1,
                        perf_mode=DR,
                    )
                    first = False
            nc.vector.tensor_copy(v_sb[mt][:, :, 0:DH], ps)

    def mk_pair(hp):
        return [(2 * hp, 0, []), (2 * hp + 1, 64, [])]

    def scores_unit(hp, qt, pair, g, hi):
        # one scores psum tile for head pair[hi], k-tiles 2g/2g+1:
        # stationary = duplicated k8 pair [64,2,128], moving = (q_hi, q_lo)
        # compensation pair [64,2,512]; exp follows immediately.
        h, po, exps = pair[hi]
        ps = scores_psum.tile([128, 1024], FP, name="ps_s", tag="scores")
        for r2 in range(2):
            j = 2 * g + r2
            nc.tensor.matmul(
                ps[:, r2 * 512 : (r2 + 1) * 512],
                lhsT=k8_sb[po : po + 64, hp, :, j * 128 : (j + 1) * 128],
                rhs=q8_sb[po : po + 64, hp, :, qt * 512 : (qt + 1) * 512],
                start=True,
                stop=True,
                perf_mode=DR,
            )
        ex = expS_pool.tile([128, 1024], F16, name="ex", tag="expS")
        nc.scalar.activation(ex, ps, AF.Exp, scale=EXP_SCALE)
        exps.append(ex)

    def ctx_q(hp, qt, pair, hi, qs):
        # flipped ctx: out[q_tile(128), 65] = sum_kt expST[kt, q].T @ [64v|64]
        # -- the streamed dim is only 65 wide (half the PE cost of the
        # [65, 512] layout) and the denominator lands as column 64, a
        # per-partition scalar: recip + tensor_scalar_mul, no broadcast.
        h, po, exps = pair[hi]
        pc = ctx_psum.tile([128, 65], FP, name="pcq", tag="ctx")
        c0 = qs * 128
        for j in range(TT):
            nc.tensor.matmul(
                pc,
                lhsT=exps[j // 2][:, (j % 2) * 512 + c0 : (j % 2) * 512 + c0 + 128],
                rhs=v_sb[j][:, h, :],
                start=(j == 0),
                stop=(j == TT - 1),
            )
        rcp = rcp_pool.tile([128, 1], FP, name="rcp", tag="rcp")
        cn = ctxN_pool.tile([128, DH], F16, name="ctxN", tag="ctxN")
        nc.vector.reciprocal(rcp, pc[:, DH : DH + 1])
        nc.vector.tensor_scalar_mul(cn, pc[:, 0:DH], rcp)
        return cn

    def ctx_t(hp, qt, cns):
        # transpose the pair's eight [q(128), dh(64)] normalized tiles back
        # into ctxT layout: PE transposes into one fp16 psum (odd head via
        # the col-64 tile position), then a single 2x-mode DVE copy.
        pt = proj_psum.tile([128, 512], F16, name="pt", tag="proj")
        for hi in range(2):
            po = 64 * hi
            for qs in range(4):
                nc.tensor.transpose(
                    pt[po : po + 64, qs * 128 : (qs + 1) * 128],
                    cns[hi][qs],
                    ident_sb,
                )
        nc.vector.tensor_copy(
            ctxT_sb[:, hp, qt * 512 : (qt + 1) * 512], pt)

    cn_store = {}

    def ctx_qu(key, hp, qt, pair, hi, qs):
        cn_store.setdefault(key, [[None] * 4 for _ in range(2)])
        cn_store[key][hi][qs] = ctx_q(hp, qt, pair, hi, qs)

    def ctx_tu(key, hp, qt):
        ctx_t(hp, qt, cn_store[key])

    def ctx_phase(key, hp, qt, pair):
        for hi in range(2):
            for qs in range(4):
                ctx_qu(key, hp, qt, pair, hi, qs)
        ctx_tu(key, hp, qt)

    def mk_ctx_units(key, hp, qt, pair):
        us = [
            (lambda hi=hi, qs=qs: ctx_qu(key, hp, qt, pair, hi, qs))
            for hi in range(2) for qs in range(4)
        ]
        us.append(lambda: ctx_tu(key, hp, qt))
        return us

    def out_proj(mts, split_dma=False):
        # out[m=t_tile(128), n=dout(384)] = sum_c ctxT16[c,m].T @ wo16[c,n];
        # the two psum->sbuf copies split across DVE and ScalarE (ACT is
        # idle by this phase) so the final DMAs unblock sooner. Tail tiles
        # borrow the (dead by then) scores psum pool for their second half
        # so the copy latency stops gating the 2-deep proj rotation.
        for mt in mts:
            osb = out_pool.tile([128, D], F16, name="osb", tag="outsb")
            for n2 in range(2):
                pool = scores_psum if (split_dma and n2 == 1) else proj_psum
                ps = pool.tile([128, 384], FP, name="ps_o", tag="proj" if pool is proj_psum else "scores")
                for c in range(MC):
                    nc.tensor.matmul(
                        ps,
                        lhsT=ctxT_sb[:, c, mt * 128 : (mt + 1) * 128],
                        rhs=wo_sb[:, c, n2 * 384 : (n2 + 1) * 384],
                        start=(c == 0),
                        stop=(c == MC - 1),
                    )
                if n2 == 0:
                    nc.vector.tensor_copy(osb[:, 0:384], ps)
                elif split_dma:
                    # tail tiles: ACT is free (exps done); halves copy in
                    # parallel on DVE + ACT so the tile's DMA unblocks fast
                    nc.scalar.copy(osb[:, 384:768], ps)
                else:
                    # early tiles: exps still stream on ACT, and gpsimd
                    # cannot read PSUM, so DVE takes both halves
                    nc.vector.tensor_copy(osb[:, 384:768], ps)
            # alternate issue sequencers so tail DMAs don't serialize
            eng = nc.sync if mt % 2 == 0 else nc.scalar
            eng.dma_start(out=out[mt * 128 : (mt + 1) * 128, :], in_=osb)

    # The scores psum pool is 2 tiles deep, so the PE scores stream runs
    # exactly ~2 exps ahead of ACT and each scores unit (2 DR matmuls,
    # ~214ns) must be followed by ~0.8us of OTHER ready PE work or PE
    # head-of-line blocks on the pool rotation. The emission is therefore
    # a 1:1 alternation of the 48 scores units with ~40 filler units
    # (projection groups / ctx chunks / out tiles), ordered so every
    # filler's dependencies are already satisfied when it pops and every
    # scores phase's q/k inputs are fully emitted before its first unit.
    p00, p01 = mk_pair(0), mk_pair(0)
    p10, p11 = mk_pair(1), mk_pair(1)
    p20, p21 = mk_pair(2), mk_pair(2)
    pcs = {k: [None, None] for k in ("00", "01", "10", "11", "20", "21")}

    fillers = [
        lambda: qk_proj(2, ns=[0], dsts=("q",)),
        lambda: qk_proj(2, ns=[0], dsts=("k",)),
        lambda: qk_proj(2, ns=[1], dsts=("k",)),
        lambda: qk_proj(2, ns=[1], dsts=("q",)),
        lambda: v_proj([0]), lambda: v_proj([1]),
        lambda: v_proj([2]), lambda: v_proj([3]),
        lambda: v_proj([4]), lambda: v_proj([5]),
        lambda: v_proj([6]), lambda: v_proj([7]),
        *mk_ctx_units("00", 0, 0, p00),
        *mk_ctx_units("01", 0, 1, p01),
        *mk_ctx_units("10", 1, 0, p10),
        lambda: ctx_qu("11", 1, 1, p11, 0, 0),
    ]

    # head: critical qk m0/n0 with fine-split copies, then the (0,0)
    # scores units; the n1-half's k groups slot in before the g2 units
    # that need them, the (not-yet-needed) q n1-half after
    qk_proj(0, ns=[0], fine=True)
    for g, hi in ((0, 0), (0, 1), (1, 0), (1, 1)):
        scores_unit(0, 0, p00, g, hi)
    qk_proj(0, ns=[1], dsts=("k",))
    scores_unit(0, 0, p00, 2, 0)
    scores_unit(0, 0, p00, 2, 1)
    qk_proj(0, ns=[1], dsts=("q",))
    scores_unit(0, 0, p00, 3, 0)
    scores_unit(0, 0, p00, 3, 1)
    qk_proj(1)

    # paired stream: 40 scores units x 40 fillers
    stream = [(0, 1, p01), (1, 0, p10), (1, 1, p11), (2, 0, p20), (2, 1, p21)]
    fi = iter(fillers)
    for hp, qt, pair in stream:
        for g in range(4):
            for hi in range(2):
                scores_unit(hp, qt, pair, g, hi)
                f = next(fi, None)
                if f is not None:
                    f()
    for f in fi:
        f()

    # drain region (the last phases' exps still streaming on ACT)
    for hi in range(2):
        for qs in range(4):
            if (hi, qs) != (0, 0):
                ctx_qu("11", 1, 1, p11, hi, qs)
    ctx_tu("11", 1, 1)
    ctx_phase("20", 2, 0, p20)
    out_proj([0])
    out_proj([1])
    out_proj([2])
    out_proj([3])
    for qs in range(4):
        ctx_qu("21", 2, 1, p21, 0, qs)
    # true tail: after the final exp only head B's four ctx units, the
    # pair's transpose + copy, and the qt1 out tiles remain
    for qs in range(4):
        ctx_qu("21", 2, 1, p21, 1, qs)
    ctx_tu("21", 2, 1)
    for mt in range(4, TT):
        out_proj([mt], split_dma=True)


_PROGRAM = None


def build_program():
    global _PROGRAM
    if _PROGRAM is not None:
        return _PROGRAM
    nc = bacc.Bacc("TRN2", target_bir_lowering=False, debug=False, num_devices=NCORES)
    xh = nc.dram_tensor("xh", (D, T), F8, kind="ExternalInput").ap()
    xl = nc.dram_tensor("xl", (D, T), F8, kind="ExternalInput").ap()
    wqh = nc.dram_tensor("wqh", (D, DPC), F8, kind="ExternalInput").ap()
    wql = nc.dram_tensor("wql", (D, DPC), F8, kind="ExternalInput").ap()
    wkh = nc.dram_tensor("wkh", (D, DPC), F8, kind="ExternalInput").ap()
    wkl = nc.dram_tensor("wkl", (D, DPC), F8, kind="ExternalInput").ap()
    wvh = nc.dram_tensor("wvh", (D, DPC), F8, kind="ExternalInput").ap()
    wvl = nc.dram_tensor("wvl", (D, DPC), F8, kind="ExternalInput").ap()
    wo = nc.dram_tensor("wo", (DPC, D), F16, kind="ExternalInput").ap()
    ones = nc.dram_tensor("ones", (128, HPC), F16, kind="ExternalInput").ap()
    ident = nc.dram_tensor("ident", (128, 128), F16, kind="ExternalInput").ap()
    out = nc.dram_tensor("out", (T, D), F16, kind="ExternalOutput").ap()
    from contextlib import ExitStack

    with TileContext(nc) as tc, ExitStack() as st:
        emit_mha(tc, xh, xl, wqh, wql, wkh, wkl, wvh, wvl, wo, ones, ident, out, st)
    nc.compile()
    _PROGRAM = nc
    return nc


def _split8(a):
    hi = np.clip(a, -240.0, 240.0).astype(E4NP)
    lo = np.clip(a - hi.astype(np.float32), -240.0, 240.0).astype(E4NP)
    return np.ascontiguousarray(hi), np.ascontiguousarray(lo)


def make_in_maps(x, Wq, Wk, Wv, Wo):
    x = np.asarray(x, dtype=np.float32)
    ones = np.full((128, HPC), ONES_VAL, np.float16)
    ident = np.eye(128, dtype=np.float16)
    xs = [_split8(x[b].T) for b in range(B)]
    in_maps = []
    for core in range(NCORES):
        b, hh = core // 2, core % 2
        sl = slice(hh * DPC, (hh + 1) * DPC)
        wqh, wql = _split8(np.asarray(Wq)[sl].T.astype(np.float32) * WSCALE)
        wkh, wkl = _split8(np.asarray(Wk)[sl].T.astype(np.float32) * WSCALE)
        wvh, wvl = _split8(np.asarray(Wv)[sl].T.astype(np.float32) * WSCALE)
        in_maps.append(
            {
                "xh": xs[b][0],
                "xl": xs[b][1],
                "wqh": wqh, "wql": wql,
                "wkh": wkh, "wkl": wkl,
                "wvh": wvh, "wvl": wvl,
                "wo": np.ascontiguousarray(np.asarray(Wo)[:, sl].T.astype(np.float16)),
                "ones": ones,
                "ident": ident,
            }
        )
    return in_maps


def kernel(x, Wq, Wk, Wv, Wo, bo):
    nc = build_program()
    in_maps = make_in_maps(x, Wq, Wk, Wv, Wo)
    res = run_bass_kernel_spmd(nc, in_maps, core_ids=list(range(NCORES)))
    bo = np.asarray(bo, dtype=np.float32)
    out = np.empty((B, T, D), dtype=np.float32)
    for b in range(B):
        out[b] = (res.results[2 * b]["out"].astype(np.float32)
                  + res.results[2 * b + 1]["out"].astype(np.float32) + bo)
    return out


# revision 87
# speedup vs baseline: 1.0098x; 1.0043x over previous
"""MultiHeadAttention Trainium2 kernel (8 NeuronCores, SPMD).

Reference computation (B=4, T=1024, D=768, H=12, Dh=64):
    q = x @ Wq.T ; k = x @ Wk.T ; v = x @ Wv.T       (per-head reshape)
    attn = softmax((q @ k.T) / 8)
    out = (attn @ v) @ Wo.T + bo

Sharding: 8 cores = 4 batches x 2 head-halves (6 heads each). Each core
computes a [1024, 768] partial of the output projection for its 6 heads;
the host sums the two partials per batch and adds the bias.

fp8 DoubleRow strategy (DR contracts [p<=128, 2] free-dim pairs at 0.5
cycles/output-column -- 4x the MAC throughput of fp32r/bf16 per pass):
  - Projections: x and W*64 pre-split on host into e4m3 (hi, lo) with
    lo = fp8(a - fp8(a)), DR-contracted over d_in chunk pairs. v uses
    three compensation terms (hi*hi + lo*hi + hi*lo, ~bf16 accuracy at
    0.75x the fp32r PE cost); q/k drop the x_lo term (0.5x cost) since
    k is requantized to single fp8 for the scores matmul anyway.
  - Scores: DR with the moving operand carrying the (q_hi, q_lo)
    compensation pair and the stationary side a duplicated single-fp8 k.
    S = k8 . (q_hi + q_lo): half the PE cost, error ~ k/x quantization
    (7.5e-3 rel fro end to end on hardware, vs the 2e-2 gate).
  - exp folds the 64*64 operand scaling away via its scale arg
    (S_psum = 32768 * logits); the ones column is 64.0 so the softmax
    denominator in the ctx matmul carries the same scale as 64*v.
  - ctx + out_proj run in fp16 (same 1.0 cycles/row, half the SBUF/DMA);
    the out partials leave as fp16 too (|out| ~ 0.1, fp16 err ~5e-4).

The ctx stage is computed in the FLIPPED layout out[q_tile(128), 65]:
the matmul cost model charges only the streamed dimension, so streaming
the 64 v columns + denominator column (65) instead of 512 q columns
halves the ctx PE cost, and the softmax denominator lands as a
per-partition scalar (DVE reciprocal + tensor_scalar_mul -- no gpsimd
partition broadcast). PE transposes (identity matmul, fp16 psum, odd
head via the col-64 tile position) rebuild the ctxT layout the output
projection needs, one [128,512] psum + single 2x-mode DVE copy per
head-pair/qt.

Schedule: the 48 scores-tile units (2 DR matmuls + 1 exp each) alternate
1:1 with filler units (projection groups, flipped-ctx units, out tiles)
in dependency order, because the 2-deep scores psum pool ties the PE
scores stream to ~2 exps ahead of ACT and PE's queue is in-order. Each
flipped-ctx unit holds all four of its head's expS tiles until it runs,
so the expS pool is 22 deep (two full phases + lead). qk m1 emits in
the head shadow and all non-critical DMA issues ride the SP sequencer
(ACT's sequencer must stay clear to decode the first exps). A
dummy-matmul chain bridges the initial DMA wait so the cost model's PE
p-state ramp elapses on throwaway work; after the final exp only head
B's four ctx units, one transpose batch, and the qt1 out tiles
(borrowing the dead scores psum banks) remain.

Per-core dataflow:
    x_hi/x_lo [768,1024] e4m3 (host-split, pretransposed) -> SBUF
    q',k' = 64*(x W) via 6 DR matmuls per [128,512] psum tile (v: 9)
        q8 = (fp8(q'), fp8(q'-fp8(q'))) pair; k8 duplicated pair
    S.T tiles [kt=128, qt=512] = DR(k8_pair, q8_pair)
    expS = exp(S.T * 1/32768) via ScalarE -> fp16
    ctx psum [q 128, 65] = expST.T @ [64v16 | 64.0]  (fp16, K=kt accum)
    ctx_norm = ctx * recip(col 64) -> fp16, PE-transposed back to ctxT
    out_partial = ctxT_norm.T @ Wo16.T   (fp16, K=384 accumulate) -> fp16
"""

import numpy as np
import ml_dtypes

import concourse.mybir as mybir
from concourse import bacc
from concourse.tile import TileContext
from concourse.bass_utils import run_bass_kernel_spmd

FP = mybir.dt.float32
F16 = mybir.dt.float16
F8 = mybir.dt.float8e4
AF = mybir.ActivationFunctionType
DR = mybir.MatmulPerfMode.DoubleRow

E4NP = ml_dtypes.float8_e4m3

B, T, D = 4, 1024, 768
H, DH = 12, 64
NCORES = 8
HPC = 6           # heads per core
DPC = HPC * DH    # 384 head-dims per core
KC = D // 128     # 6 contraction chunks for d_in
CP = KC // 2      # 3 chunk-pairs for DoubleRow
MC = DPC // 128   # 3 chunks of per-core head dims
NT = T // 512     # 2 free-dim tiles of tokens
TT = T // 128     # 8 partition tiles of tokens

WSCALE = 64.0             # host scale on Wq/Wk/Wv (fp8 range usage)
EXP_SCALE = 1.0 / 32768.0  # exp reads S_psum = 64q . 64k = 32768*(qk/8)
ONES_VAL = 64.0           # denominator column matches the 64*v scale


def emit_mha(tc, xh, xl, wqh, wql, wkh, wkl, wvh, wvl, wo, ones, ident, out, ctx):
    nc = tc.nc

    singles = ctx.enter_context(tc.tile_pool(name="singles", bufs=1))
    proj_psum = ctx.enter_context(tc.tile_pool(name="proj_psum", bufs=2, space="PSUM"))
    scores_psum = ctx.enter_context(
        tc.tile_pool(name="scores_psum", bufs=2, space="PSUM")
    )
    ctx_psum = ctx.enter_context(tc.tile_pool(name="ctx_psum", bufs=2, space="PSUM"))
    expS_pool = ctx.enter_context(tc.tile_pool(name="expS", bufs=22))
    rcp_pool = ctx.enter_context(tc.tile_pool(name="rcp", bufs=10))
    ctxN_pool = ctx.enter_context(tc.tile_pool(name="ctxN", bufs=12))
    out_pool = ctx.enter_context(tc.tile_pool(name="outsb", bufs=6))

    # ---------------- staged input DMAs ----------------
    xh_sb = singles.tile([128, KC, T], F8, name="xh_sb", tag="xh_sb")
    xl_sb = singles.tile([128, KC, T], F8, name="xl_sb", tag="xl_sb")
    xhr = xh.rearrange("(c p) t -> p c t", p=128)
    xlr = xl.rearrange("(c p) t -> p c t", p=128)
    w_sb = {}
    w_r = {}
    for nm, t in (("qh", wqh), ("ql", wql), ("kh", wkh), ("kl", wkl),
                  ("vh", wvh), ("vl", wvl)):
        w_sb[nm] = singles.tile([128, KC, DPC], F8, name=f"w{nm}", tag=f"w{nm}")
        w_r[nm] = t.rearrange("(c p) d -> p c d", p=128)
    wo_sb = singles.tile([128, MC, D], F16, name="wo_sb", tag="wo_sb")

    # Few, large DMAs (each dma_start costs ~600ns of serial issue time on
    # its engine's sequencer). The qk-m0/n0-critical transfers are split
    # across the idle sequencers so they all issue within ~2 rounds; the x
    # halves are further split so their transfer time halves.
    # The transfer device is effectively serial in the cost model, so the
    # queue ORDER is the arrival order: everything the first two psum
    # groups need goes first, ordered by when the group's matmuls touch
    # it (x_lo last: the hi*lo/lo*hi terms are emitted last). Issue
    # alternates SP/ACT sequencers to keep the queue fed.
    ones_sb = singles.tile([128, HPC], F16, name="ones_sb", tag="ones_sb")
    # with 2-term q/k, x_lo is only needed by the v projection -- the
    # critical head chain is just x_hi n0 + the four qk m0 weight slices
    nc.sync.dma_start(out=xh_sb[:, :, 0:512], in_=xhr[:, :, 0:512])
    nc.scalar.dma_start(out=w_sb["qh"][:, :, 0:128], in_=w_r["qh"][:, :, 0:128])
    nc.sync.dma_start(out=w_sb["ql"][:, :, 0:128], in_=w_r["ql"][:, :, 0:128])
    nc.scalar.dma_start(out=w_sb["kh"][:, :, 0:128], in_=w_r["kh"][:, :, 0:128])
    nc.sync.dma_start(out=w_sb["kl"][:, :, 0:128], in_=w_r["kl"][:, :, 0:128])
    nc.scalar.dma_start(out=xh_sb[:, :, 512:1024], in_=xhr[:, :, 512:1024])
    nc.gpsimd.dma_start(out=ones_sb, in_=ones)
    # everything below is issue-latency-insensitive: it all goes on the SP
    # sequencer so the ACT sequencer is free to decode the first exps
    # (each dma_start occupies its sequencer ~650ns, and ACT's queue sits
    # ahead of every activation instruction)
    for nm in ("kh", "qh", "kl", "ql"):
        nc.sync.dma_start(out=w_sb[nm][:, :, 128:256], in_=w_r[nm][:, :, 128:256])
    for nm in ("kh", "qh", "kl", "ql"):
        nc.sync.dma_start(out=w_sb[nm][:, :, 256:DPC], in_=w_r[nm][:, :, 256:DPC])
    nc.sync.dma_start(out=w_sb["vh"], in_=w_r["vh"])
    nc.sync.dma_start(out=w_sb["vl"], in_=w_r["vl"])
    nc.sync.dma_start(out=xl_sb[:, :, 0:512], in_=xlr[:, :, 0:512])
    nc.sync.dma_start(out=xl_sb[:, :, 512:1024], in_=xlr[:, :, 512:1024])
    nc.sync.dma_start(out=wo_sb, in_=wo.rearrange("(c p) d -> p c d", p=128))
    ident_sb = singles.tile([128, 128], F16, name="ident_sb", tag="ident_sb")
    nc.sync.dma_start(out=ident_sb, in_=ident)

    # warm-up: a chain of dummy matmuls keeps PE continuously busy from
    # ~1.2us until the first inputs land (~4.5us), so the cost model's
    # 3us p-state ramp elapses on throwaway work. The ramp clock resets
    # whenever PE goes idle, so the chain must bridge the whole DMA wait.
    wu_sb = singles.tile([128, 256], F16, name="wu_sb", tag="wu_sb")
    nc.vector.memset(wu_sb, 0.0)
    for _ in range(5):
        ps_wu = proj_psum.tile([128, 512], FP, name="ps_wu", tag="proj")
        nc.tensor.matmul(ps_wu[:, 0:256], lhsT=wu_sb[:, 0:128],
                         rhs=wu_sb[:, 0:256], start=True, stop=True)
        nc.tensor.matmul(ps_wu[:, 256:512], lhsT=wu_sb[:, 0:128],
                         rhs=wu_sb[:, 0:256], start=True, stop=True)

    # q8: (hi, lo) compensation pair; k8: duplicated single-fp8 pair
    q8_sb = singles.tile([128, MC, 2, T], F8, name="q8_sb", tag="q8_sb")
    k8_sb = singles.tile([128, MC, 2, T], F8, name="k8_sb", tag="k8_sb")
    ctxT_sb = singles.tile([128, MC, T], F16, name="ctxT_sb", tag="ctxT_sb")

    # v tiles [t_tile, 6 heads x (64 v cols + ones col)]: the 64.0 column
    # makes each head's ctx matmul also produce its softmax denominator
    # (psum row 64) at the same 64x scale as v. One early DMA stages the
    # column; gpsimd copies fan it out (NOT on DVE, whose in-order queue
    # must stay clear for the q8/k8 psum copies).
    v_sb = []
    for i in range(TT):
        vt = singles.tile([128, HPC, DH + 1], F16, name=f"v_sb{i}", tag=f"v_sb{i}")
        nc.gpsimd.tensor_copy(vt[:, :, DH : DH + 1], ones_sb)
        v_sb.append(vt)

    # DR terms of the hi/lo-compensated product, as (x, w) suffixes.
    # v keeps all three terms; q/k drop the x_lo correction -- k is about
    # to be requantized to single fp8 for the scores matmul anyway and q's
    # error stays ~fp8-lo sized (1.1e-2 rel fro end to end, vs 2e-2 gate).
    TERMS3 = (("h", "h"), ("h", "l"), ("l", "h"))
    TERMS2 = (("h", "h"), ("h", "l"))

    def qk_proj(m, ns=range(NT), dsts=("q", "k"), fine=False, k_on_act=False):
        # q'/k' chunk m: psum[m=dout(128), n=t(512)] = 64 * sum_c w[c].T x[c]
        # 9 DR matmuls: 3 hi/lo terms x 3 chunk-pairs, each contracting 256
        xs = {"h": xh_sb, "l": xl_sb}
        paths = {"k": (("kh", "kl"), k8_sb, False), "q": (("qh", "ql"), q8_sb, True)}
        for n in ns:
            for wk_, dst, is_q in (paths[d] for d in dsts):
                ps = proj_psum.tile([128, 512], FP, name="ps_qk", tag="proj")
                first = True
                for xsfx, wsfx in TERMS2:
                    wt = w_sb[wk_[0][0] + wsfx]
                    xt = xs[xsfx]
                    for cp in range(CP):
                        nc.tensor.matmul(
                            ps,
                            lhsT=wt[:, 2 * cp : 2 * cp + 2, m * 128 : (m + 1) * 128],
                            rhs=xt[:, 2 * cp : 2 * cp + 2, n * 512 : (n + 1) * 512],
                            start=first,
                            stop=(xsfx, wsfx) == TERMS2[-1] and cp == CP - 1,
                            perf_mode=DR,
                        )
                        first = False
                # fine=True splits the k copies in half-widths so the first
                # scores tiles (which only need k columns 0:256) unblock
                # earlier -- used for the critical m0/n0 group only. The q
                # copies stay full-width (every scores tile reads all 512).
                cw = 256 if (fine and not is_q) else 512
                for c0 in range(n * 512, (n + 1) * 512, cw):
                    csl = slice(c0, c0 + cw)
                    psl = slice(c0 - n * 512, c0 - n * 512 + cw)
                    if is_q:
                        nc.vector.tensor_copy(dst[:, m, 0, csl], ps[:, psl])
                        nc.vector.tensor_sub(dst[:, m, 1, csl], ps[:, psl],
                                             dst[:, m, 0, csl])
                    elif k_on_act:
                        # ACT is idle during the head; copying k there runs
                        # in parallel with the q copies on DVE
                        nc.scalar.copy(dst[:, m, 0, csl], ps[:, psl])
                        nc.scalar.copy(dst[:, m, 1, csl], ps[:, psl])
                    else:
                        nc.vector.tensor_copy(dst[:, m, 0, csl], ps[:, psl])
                        nc.vector.tensor_copy(dst[:, m, 1, csl], ps[:, psl])

    def v_proj(mts=range(TT)):
        # v': psum[m=t_tile(128), n=dh(384)] = 64 * sum_c x[c,m].T wv[c,n]
        xs = {"h": xh_sb, "l": xl_sb}
        for mt in mts:
            ps = proj_psum.tile([128, DPC], FP, name="ps_v", tag="proj")
            first = True
            for xsfx, wsfx in TERMS3:
                xt = xs[xsfx]
                wt = w_sb["v" + wsfx]
                for cp in range(CP):
                    nc.tensor.matmul(
                        ps,
                        lhsT=xt[:, 2 * cp : 2 * cp + 2, mt * 128 : (mt + 1) * 128],
                        rhs=wt[:, 2 * cp : 2 * cp + 2, :],
                        start=first,
                        stop=(xsfx, wsfx) == TERMS3[-1] and cp == CP - 1,
                        perf_mode=DR,
                    )
                    first = False
            nc.vector.tensor_copy(v_sb[mt][:, :, 0:DH], ps)

    def mk_pair(hp):
        return [(2 * hp, 0, []), (2 * hp + 1, 64, [])]

    def scores_unit(hp, qt, pair, g, hi):
        # one scores psum tile for head pair[hi], k-tiles 2g/2g+1:
        # stationary = duplicated k8 pair [64,2,128], moving = (q_hi, q_lo)
        # compensation pair [64,2,512]; exp follows immediately.
        h, po, exps = pair[hi]
        ps = scores_psum.tile([128, 1024], FP, name="ps_s", tag="scores")
        for r2 in range(2):
            j = 2 * g + r2
            nc.tensor.matmul(
                ps[:, r2 * 512 : (r2 + 1) * 512],
                lhsT=k8_sb[po : po + 64, hp, :, j * 128 : (j + 1) * 128],
                rhs=q8_sb[po : po + 64, hp, :, qt * 512 : (qt + 1) * 512],
                start=True,
                stop=True,
                perf_mode=DR,
            )
        ex = expS_pool.tile([128, 1024], F16, name="ex", tag="expS")
        nc.scalar.activation(ex, ps, AF.Exp, scale=EXP_SCALE)
        exps.append(ex)

    def ctx_q(hp, qt, pair, hi, qs):
        # flipped ctx: out[q_tile(128), 65] = sum_kt expST[kt, q].T @ [64v|64]
        # -- the streamed dim is only 65 wide (half the PE cost of the
        # [65, 512] layout) and the denominator lands as column 64, a
        # per-partition scalar: recip + tensor_scalar_mul, no broadcast.
        h, po, exps = pair[hi]
        pc = ctx_psum.tile([128, 65], FP, name="pcq", tag="ctx")
        c0 = qs * 128
        for j in range(TT):
            nc.tensor.matmul(
                pc,
                lhsT=exps[j // 2][:, (j % 2) * 512 + c0 : (j % 2) * 512 + c0 + 128],
                rhs=v_sb[j][:, h, :],
                start=(j == 0),
                stop=(j == TT - 1),
            )
        rcp = rcp_pool.tile([128, 1], FP, name="rcp", tag="rcp")
        cn = ctxN_pool.tile([128, DH], F16, name="ctxN", tag="ctxN")
        nc.vector.reciprocal(rcp, pc[:, DH : DH + 1])
        nc.vector.tensor_scalar_mul(cn, pc[:, 0:DH], rcp)
        return cn

    def ctx_t(hp, qt, cns):
        # transpose the pair's eight [q(128), dh(64)] normalized tiles back
        # into ctxT layout: PE transposes into one fp16 psum (odd head via
        # the col-64 tile position), then a single 2x-mode DVE copy.
        pt = proj_psum.tile([128, 512], F16, name="pt", tag="proj")
        for hi in range(2):
            po = 64 * hi
            for qs in range(4):
                nc.tensor.transpose(
                    pt[po : po + 64, qs * 128 : (qs + 1) * 128],
                    cns[hi][qs],
                    ident_sb,
                )
        nc.vector.tensor_copy(
            ctxT_sb[:, hp, qt * 512 : (qt + 1) * 512], pt)

    cn_store = {}

    def ctx_qu(key, hp, qt, pair, hi, qs):
        cn_store.setdefault(key, [[None] * 4 for _ in range(2)])
        cn_store[key][hi][qs] = ctx_q(hp, qt, pair, hi, qs)

    def ctx_tu(key, hp, qt):
        ctx_t(hp, qt, cn_store[key])

    def ctx_phase(key, hp, qt, pair):
        for hi in range(2):
            for qs in range(4):
                ctx_qu(key, hp, qt, pair, hi, qs)
        ctx_tu(key, hp, qt)

    def mk_ctx_units(key, hp, qt, pair):
        us = [
            (lambda hi=hi, qs=qs: ctx_qu(key, hp, qt, pair, hi, qs))
            for hi in range(2) for qs in range(4)
        ]
        us.append(lambda: ctx_tu(key, hp, qt))
        return us

    def out_proj(mts, split_dma=False):
        # out[m=t_tile(128), n=dout(384)] = sum_c ctxT16[c,m].T @ wo16[c,n];
        # the two psum->sbuf copies split across DVE and ScalarE (ACT is
        # idle by this phase) so the final DMAs unblock sooner. Tail tiles
        # borrow the (dead by then) scores psum pool for their second half
        # so the copy latency stops gating the 2-deep proj rotation.
        for mt in mts:
            osb = out_pool.tile([128, D], F16, name="osb", tag="outsb")
            for n2 in range(2):
                pool = scores_psum if (split_dma and n2 == 1) else proj_psum
                ps = pool.tile([128, 384], FP, name="ps_o", tag="proj" if pool is proj_psum else "scores")
                for c in range(MC):
                    nc.tensor.matmul(
                        ps,
                        lhsT=ctxT_sb[:, c, mt * 128 : (mt + 1) * 128],
                        rhs=wo_sb[:, c, n2 * 384 : (n2 + 1) * 384],
                        start=(c == 0),
                        stop=(c == MC - 1),
                    )
                if n2 == 0:
                    nc.vector.tensor_copy(osb[:, 0:384], ps)
                elif split_dma:
                    # tail tiles: ACT is free (exps done); halves copy in
                    # parallel on DVE + ACT so the tile's DMA unblocks fast
                    nc.scalar.copy(osb[:, 384:768], ps)
                else:
                    # early tiles: exps still stream on ACT, and gpsimd
                    # cannot read PSUM, so DVE takes both halves
                    nc.vector.tensor_copy(osb[:, 384:768], ps)
            # early tiles alternate issue sequencers; tail tiles all ride
            # SP -- the scalar sequencer is busy with the n2=1 copies there
            # and would delay the last DMAs behind them
            eng = nc.sync if (split_dma or mt % 2 == 0) else nc.scalar
            eng.dma_start(out=out[mt * 128 : (mt + 1) * 128, :], in_=osb)

    # The scores psum pool is 2 tiles deep, so the PE scores stream runs
    # exactly ~2 exps ahead of ACT and each scores unit (2 DR matmuls,
    # ~214ns) must be followed by ~0.8us of OTHER ready PE work or PE
    # head-of-line blocks on the pool rotation. The emission is therefore
    # a 1:1 alternation of the 48 scores units with ~40 filler units
    # (projection groups / ctx chunks / out tiles), ordered so every
    # filler's dependencies are already satisfied when it pops and every
    # scores phase's q/k inputs are fully emitted before its first unit.
    p00, p01 = mk_pair(0), mk_pair(0)
    p10, p11 = mk_pair(1), mk_pair(1)
    p20, p21 = mk_pair(2), mk_pair(2)
    pcs = {k: [None, None] for k in ("00", "01", "10", "11", "20", "21")}

    fillers = [
        lambda: qk_proj(2, ns=[0], dsts=("q",)),
        lambda: qk_proj(2, ns=[0], dsts=("k",)),
        lambda: qk_proj(2, ns=[1], dsts=("k",)),
        lambda: qk_proj(2, ns=[1], dsts=("q",)),
        lambda: v_proj([0]), lambda: v_proj([1]),
        lambda: v_proj([2]), lambda: v_proj([3]),
        lambda: v_proj([4]), lambda: v_proj([5]),
        lambda: v_proj([6]), lambda: v_proj([7]),
        *mk_ctx_units("00", 0, 0, p00),
        *mk_ctx_units("01", 0, 1, p01),
        *mk_ctx_units("10", 1, 0, p10),
        lambda: ctx_qu("11", 1, 1, p11, 0, 0),
    ]

    # head: critical qk m0/n0 with fine-split copies, then the (0,0)
    # scores units; the n1-half's k groups slot in before the g2 units
    # that need them, the (not-yet-needed) q n1-half after
    qk_proj(0, ns=[0], fine=True)
    for g, hi in ((0, 0), (0, 1), (1, 0), (1, 1)):
        scores_unit(0, 0, p00, g, hi)
    qk_proj(0, ns=[1], dsts=("k",))
    scores_unit(0, 0, p00, 2, 0)
    scores_unit(0, 0, p00, 2, 1)
    qk_proj(0, ns=[1], dsts=("q",))
    scores_unit(0, 0, p00, 3, 0)
    scores_unit(0, 0, p00, 3, 1)
    qk_proj(1)

    # paired stream: 40 scores units x 40 fillers
    stream = [(0, 1, p01), (1, 0, p10), (1, 1, p11), (2, 0, p20), (2, 1, p21)]
    fi = iter(fillers)
    for hp, qt, pair in stream:
        for g in range(4):
            for hi in range(2):
                scores_unit(hp, qt, pair, g, hi)
                f = next(fi, None)
                if f is not None:
                    f()
    for f in fi:
        f()

    # drain region (the last phases' exps still streaming on ACT)
    for hi in range(2):
        for qs in range(4):
            if (hi, qs) != (0, 0):
                ctx_qu("11", 1, 1, p11, hi, qs)
    ctx_tu("11", 1, 1)
    ctx_phase("20", 2, 0, p20)
    out_proj([0])
    out_proj([1])
    out_proj([2])
    out_proj([3])
    for qs in range(4):
        ctx_qu("21", 2, 1, p21, 0, qs)
    # true tail: after the final exp only head B's four ctx units, the
    # pair's transpose + copy, and the qt1 out tiles remain
    for qs in range(4):
        ctx_qu("21", 2, 1, p21, 1, qs)
    ctx_tu("21", 2, 1)
    for mt in range(4, TT):
        out_proj([mt], split_dma=True)


_PROGRAM = None


def build_program():
    global _PROGRAM
    if _PROGRAM is not None:
        return _PROGRAM
    nc = bacc.Bacc("TRN2", target_bir_lowering=False, debug=False, num_devices=NCORES)
    xh = nc.dram_tensor("xh", (D, T), F8, kind="ExternalInput").ap()
    xl = nc.dram_tensor("xl", (D, T), F8, kind="ExternalInput").ap()
    wqh = nc.dram_tensor("wqh", (D, DPC), F8, kind="ExternalInput").ap()
    wql = nc.dram_tensor("wql", (D, DPC), F8, kind="ExternalInput").ap()
    wkh = nc.dram_tensor("wkh", (D, DPC), F8, kind="ExternalInput").ap()
    wkl = nc.dram_tensor("wkl", (D, DPC), F8, kind="ExternalInput").ap()
    wvh = nc.dram_tensor("wvh", (D, DPC), F8, kind="ExternalInput").ap()
    wvl = nc.dram_tensor("wvl", (D, DPC), F8, kind="ExternalInput").ap()
    wo = nc.dram_tensor("wo", (DPC, D), F16, kind="ExternalInput").ap()
    ones = nc.dram_tensor("ones", (128, HPC), F16, kind="ExternalInput").ap()
    ident = nc.dram_tensor("ident", (128, 128), F16, kind="ExternalInput").ap()
    out = nc.dram_tensor("out", (T, D), F16, kind="ExternalOutput").ap()
    from contextlib import ExitStack

    with TileContext(nc) as tc, ExitStack() as st:
        emit_mha(tc, xh, xl, wqh, wql, wkh, wkl, wvh, wvl, wo, ones, ident, out, st)
    nc.compile()
    _PROGRAM = nc
    return nc


def _split8(a):
    hi = np.clip(a, -240.0, 240.0).astype(E4NP)
    lo = np.clip(a - hi.astype(np.float32), -240.0, 240.0).astype(E4NP)
    return np.ascontiguousarray(hi), np.ascontiguousarray(lo)


def make_in_maps(x, Wq, Wk, Wv, Wo):
    x = np.asarray(x, dtype=np.float32)
    ones = np.full((128, HPC), ONES_VAL, np.float16)
    ident = np.eye(128, dtype=np.float16)
    xs = [_split8(x[b].T) for b in range(B)]
    in_maps = []
    for core in range(NCORES):
        b, hh = core // 2, core % 2
        sl = slice(hh * DPC, (hh + 1) * DPC)
        wqh, wql = _split8(np.asarray(Wq)[sl].T.astype(np.float32) * WSCALE)
        wkh, wkl = _split8(np.asarray(Wk)[sl].T.astype(np.float32) * WSCALE)
        wvh, wvl = _split8(np.asarray(Wv)[sl].T.astype(np.float32) * WSCALE)
        in_maps.append(
            {
                "xh": xs[b][0],
                "xl": xs[b][1],
                "wqh": wqh, "wql": wql,
                "wkh": wkh, "wkl": wkl,
                "wvh": wvh, "wvl": wvl,
                "wo": np.ascontiguousarray(np.asarray(Wo)[:, sl].T.astype(np.float16)),
                "ones": ones,
                "ident": ident,
            }
        )
    return in_maps


def kernel(x, Wq, Wk, Wv, Wo, bo):
    nc = build_program()
    in_maps = make_in_maps(x, Wq, Wk, Wv, Wo)
    res = run_bass_kernel_spmd(nc, in_maps, core_ids=list(range(NCORES)))
    bo = np.asarray(bo, dtype=np.float32)
    out = np.empty((B, T, D), dtype=np.float32)
    for b in range(B):
        out[b] = (res.results[2 * b]["out"].astype(np.float32)
                  + res.results[2 * b + 1]["out"].astype(np.float32) + bo)
    return out
